# revision 17
# baseline (speedup 1.0000x reference)
"""Multi-head attention (B=4, S=1024, HID=1024, NH=16) on 8 trn2 NeuronCores.

Sharding: core c handles batch group bg=c//4 (2 batches) and head group
hg=c%4 (4 heads, i.e. dim slice hg*256:(hg+1)*256).  Each core computes a
partial output y_c = o_heads @ Wo[:, slice].T of the full [2048, 1024] shape;
the host sums the 4 partials per batch group and adds the (bo + bv @ Wo.T)
correction row (softmax rows sum to 1, so the V bias commutes to the end).

Device dataflow (per core, all matmul operands fp16, fp32 PSUM accum):
  p1: qT/kT = W @ x.T (transposed layout, per-partition bias via ACT/DVE),
      v = x @ Wv.T with a ones column appended (softmax denominator rides
      the PV matmul as output row 64).
  p2: per (head-pair, q-tile, k-tile): scores via two K=64 matmuls at base
      partitions 0/64 (auto row tile_position -> they run concurrently in
      the PE array); exp on ACT over [128,1024] PSUM (both heads at once);
      e = exp(s) * expb on DVE in 2x mode (expb = exp(attn_bias) is
      precomputed on the host, fp16, so the bias-add becomes an all-16-bit
      SBUF multiply instead of a 1x f32 PSUM add); PV accumulates
      [65, 512] over k-tiles; normalize via DVE reciprocal + GpSimd
      partition_broadcast + DVE multiply.
  p3: y = oT.T @ woT, staged in SBUF, one 2MB DMA per batch (fp16 partials
      summed on the host).

Phase-interleaved emission: phase 2 is ACT-exp-bound, so the other batch's
projection/output-projection work is injected into phase 2's instruction
streams at 8 points, keeping every engine busy:
    p1(0) | p2(0)+p1(1) chunks | p2(1)+p3(0) chunks | p3(1)
All DMA is MB-scale (14 transfers/core): exp-bias 4MB per head-pair, x 2MB
per batch, y 2MB per batch.  Measured ~97us/core on trn2 (differential
repeat-NEFF timing), vs ~259us for the naive per-phase version.
"""

import numpy as np

B, S, HID, NH, DK = 4, 1024, 1024, 16, 64
SCALE = DK**-0.5
P = 128
NCORES = 8
HPC = 4  # heads per core
BPC = 2  # batches per core
SL = BPC * S  # 2048 local rows
DPC = HPC * DK  # 256 local head dims
KT = HID // P  # 8 contraction tiles for the projections
ST = S // P  # 8 seq tiles of 128

_NC = None


def _build_nc(repeat=1):
    import concourse.tile as tile
    from concourse import bacc, mybir
    from contextlib import ExitStack

    f32 = mybir.dt.float32
    f16 = mybir.dt.float16
    Alu = mybir.AluOpType
    Act = mybir.ActivationFunctionType

    nc = bacc.Bacc()

    xT_d = nc.dram_tensor("xT", [HID, SL], f16, kind="ExternalInput")
    wqT_d = nc.dram_tensor("wqT", [HID, DPC], f16, kind="ExternalInput")
    wkT_d = nc.dram_tensor("wkT", [HID, DPC], f16, kind="ExternalInput")
    wvT_d = nc.dram_tensor("wvT", [HID, DPC], f16, kind="ExternalInput")
    bqk_d = nc.dram_tensor("bqk", [2, DPC], f32, kind="ExternalInput")
    woT_d = nc.dram_tensor("woT", [DPC, HID], f16, kind="ExternalInput")
    # exp(bias), transposed+tiled on host: (b, hp, kt, p, sub, q)
    ebias_d = nc.dram_tensor(
        "ebias", [BPC, HPC // 2, ST, P, 2, S], f16, kind="ExternalInput"
    )
    y_d = nc.dram_tensor("y", [SL, HID], f16, kind="ExternalOutput")

    with tile.TileContext(nc) as tc:
        with ExitStack() as ctx:
            const = ctx.enter_context(tc.tile_pool(name="const", bufs=1))
            persist = ctx.enter_context(tc.tile_pool(name="persist", bufs=1))
            xchunk = ctx.enter_context(tc.tile_pool(name="xchunk", bufs=2))
            ebp = ctx.enter_context(tc.tile_pool(name="ebp", bufs=2))
            epool = ctx.enter_context(tc.tile_pool(name="epool", bufs=3))
            e2pool = ctx.enter_context(tc.tile_pool(name="e2pool", bufs=3))
            small = ctx.enter_context(tc.tile_pool(name="small", bufs=4))
            youtp = ctx.enter_context(tc.tile_pool(name="youtp", bufs=2))
            ps_a = ctx.enter_context(tc.tile_pool(name="ps_a", bufs=3, space="PSUM"))
            ps_pv = ctx.enter_context(tc.tile_pool(name="ps_pv", bufs=2, space="PSUM"))

            # ---- constants (wq first so the first QK matmul group can start
            # as soon as wq + the first xc half land) ----
            wq_sb = const.tile([P, KT, DPC], f16, tag="wq")
            wk_sb = const.tile([P, KT, DPC], f16, tag="wk")
            wv_sb = const.tile([P, KT, DPC], f16, tag="wv")
            wo_sb = const.tile([P, 2, HID], f16, tag="wo")
            bq_sb = const.tile([P, 2], f32, tag="bq")
            bk_sb = const.tile([P, 2], f32, tag="bk")
            xT_r = xT_d.rearrange("(kt p) s -> p kt s", p=P)
            nc.sync.dma_start(wq_sb[:], wqT_d.rearrange("(kt p) m -> p kt m", p=P))

            def late_consts():
                nc.sync.dma_start(bq_sb[:], bqk_d[0].rearrange("(m p) -> p m", p=P))
                nc.sync.dma_start(bk_sb[:], bqk_d[1].rearrange("(m p) -> p m", p=P))
                nc.sync.dma_start(wk_sb[:], wkT_d.rearrange("(kt p) m -> p kt m", p=P))
                nc.sync.dma_start(wv_sb[:], wvT_d.rearrange("(kt p) m -> p kt m", p=P))
                nc.sync.dma_start(wo_sb[:], woT_d.rearrange("(kt p) n -> p kt n", p=P))

            def emit(first):
                # ---- persistent activations, per batch ----
                qT_sb, kT_sb, v_sb, oT_sb, xc_sb = [], [], [], [], [None, None]
                for b in range(BPC):
                    q_t = persist.tile([P, 2, S], f16, tag=f"qT{b}", name="q_t")
                    k_t = persist.tile([P, 2, S], f16, tag=f"kT{b}", name="k_t")
                    v_t = persist.tile(
                        [P, ST, HPC, DK + 1], f16, tag=f"v{b}", name="v_t"
                    )
                    o_t = persist.tile([P, 2, S], f16, tag=f"oT{b}", name="o_t")
                    nc.vector.memset(v_t[:, :, :, DK : DK + 1], 1.0)
                    qT_sb.append(q_t)
                    kT_sb.append(k_t)
                    v_sb.append(v_t)
                    oT_sb.append(o_t)

                def p1_prefetch(b, split_first=False):
                    xc = xchunk.tile([P, KT, S], f16, tag="xc", name="xc")
                    for h in range(2):
                        nc.sync.dma_start(
                            xc[:, :, h * 512 : (h + 1) * 512],
                            xT_r[:, :, b * S + h * 512 : b * S + (h + 1) * 512],
                        )
                    xc_sb[b] = xc

                def p1_qk_m(b, w_sb, b_sb, dst, m, on_act):
                    xc = xc_sb[b]
                    ps = ps_a.tile([P, 1024], f32, tag="a", name="a")
                    for h in range(2):
                        for kt in range(KT):
                            nc.tensor.matmul(
                                ps[:, h * 512 : (h + 1) * 512],
                                lhsT=w_sb[:, kt, m * P : (m + 1) * P],
                                rhs=xc[:, kt, h * 512 : (h + 1) * 512],
                                start=(kt == 0),
                                stop=(kt == KT - 1),
                            )
                    if on_act:
                        nc.scalar.activation(
                            dst[:, m, :], ps[:], Act.Identity, bias=b_sb[:, m : m + 1]
                        )
                    else:
                        nc.vector.tensor_scalar_add(
                            dst[:, m, :], ps[:], b_sb[:, m : m + 1]
                        )

                def p1_v(b, sts):
                    xc = xc_sb[b]
                    for st in sts:
                        ps = ps_a.tile([P, 1024], f32, tag="a", name="a")
                        for kt in range(KT):
                            nc.tensor.matmul(
                                ps[:, :DPC],
                                lhsT=xc[:, kt, st * P : (st + 1) * P],
                                rhs=wv_sb[:, kt, :],
                                start=(kt == 0),
                                stop=(kt == KT - 1),
                            )
                        nc.vector.tensor_copy(
                            out=v_sb[b][:, st, :, 0:DK],
                            in_=ps[:, :DPC].rearrange("p (h d) -> p h d", h=HPC),
                        )

                def p1_chunks(b, on_act, interleaved=False):
                    qk = [
                        lambda: p1_qk_m(b, wq_sb, bq_sb, qT_sb[b], 0, on_act),
                        lambda: p1_qk_m(b, wq_sb, bq_sb, qT_sb[b], 1, on_act),
                        lambda: p1_qk_m(b, wk_sb, bk_sb, kT_sb[b], 0, on_act),
                        lambda: p1_qk_m(b, wk_sb, bk_sb, kT_sb[b], 1, on_act),
                    ]
                    v = [
                        lambda: p1_v(b, range(0, 2)),
                        lambda: p1_v(b, range(2, 4)),
                        lambda: p1_v(b, range(4, 6)),
                        lambda: p1_v(b, range(6, 8)),
                    ]
                    return qk + v

                def p3_sl(b, ysb, sl, on_act):
                    yp = ps_a.tile([P, 1024], f32, tag="a", name="a")
                    for nt in range(2):
                        for hp in range(2):
                            nc.tensor.matmul(
                                yp[:, nt * 512 : (nt + 1) * 512],
                                lhsT=oT_sb[b][:, hp, sl * P : (sl + 1) * P],
                                rhs=wo_sb[:, hp, nt * 512 : (nt + 1) * 512],
                                start=(hp == 0),
                                stop=(hp == 1),
                            )
                    if on_act:
                        nc.scalar.copy(ysb[:, sl, :], yp[:])
                    else:
                        nc.vector.tensor_copy(out=ysb[:, sl, :], in_=yp[:])

                def p3_chunks(b, act_half):
                    holder = {}

                    def chunk(i):
                        def run():
                            if i == 0:
                                holder["ysb"] = youtp.tile(
                                    [P, ST, HID], f16, tag="yt", name="yt"
                                )
                            ysb = holder["ysb"]
                            on_act = act_half and (i % 2 == 0)
                            p3_sl(b, ysb, i, on_act)
                            if i == ST - 1:
                                nc.sync.dma_start(
                                    y_d[b * S : (b + 1) * S, :].rearrange(
                                        "(sl p) n -> p sl n", p=P
                                    ),
                                    ysb[:],
                                )

                        return run

                    return [chunk(i) for i in range(ST)]

                def phase2(b, inject):
                    ii = [0]

                    def do_inject():
                        if inject is not None and ii[0] < len(inject):
                            inject[ii[0]]()
                            ii[0] += 1

                    for hp in range(2):  # head pair (partition halves)
                        eb = ebp.tile([P, ST, 2, S], f16, tag="eb", name="eb")
                        nc.sync.dma_start(
                            eb[:], ebias_d[b, hp].rearrange("kt p s q -> p kt s q")
                        )
                        for qt in range(2):  # 512-wide q tiles
                            qlo = qt * 512
                            ovs = []
                            for sub in range(2):
                                ov = ps_pv.tile([P, 512], f32, tag="pv", name="pv")
                                ovs.append(ov)
                            for kt in range(ST):
                                if kt == ST // 2:
                                    do_inject()
                                klo = kt * P
                                sc = ps_a.tile([P, 2, 512], f32, tag="a", name="a")
                                for sub in range(2):
                                    lo = sub * DK
                                    nc.tensor.matmul(
                                        sc[:, sub, :],
                                        lhsT=kT_sb[b][lo : lo + DK, hp, klo : klo + P],
                                        rhs=qT_sb[b][lo : lo + DK, hp, qlo : qlo + 512],
                                        start=True,
                                        stop=True,
                                    )
                                er = epool.tile([P, 2, 512], f16, tag="e", name="e")
                                nc.scalar.activation(er[:], sc[:], Act.Exp)
                                et = e2pool.tile([P, 2, 512], f16, tag="e2", name="e2")
                                nc.vector.tensor_tensor(
                                    et[:],
                                    er[:],
                                    eb[:, kt, :, qlo : qlo + 512],
                                    Alu.mult,
                                )
                                for sub in range(2):
                                    h = hp * 2 + sub
                                    nc.tensor.matmul(
                                        ovs[sub][0 : DK + 1, :],
                                        lhsT=v_sb[b][:, kt, h, :],
                                        rhs=et[:, sub, :],
                                        start=(kt == 0),
                                        stop=(kt == ST - 1),
                                    )
                            for sub in range(2):
                                lo = sub * DK
                                ov = ovs[sub]
                                rr = small.tile([1, 512], f32, tag="rr", name="rr")
                                nc.vector.reciprocal(rr[:], ov[DK : DK + 1, :])
                                bc = small.tile([DK, 512], f32, tag="bc", name="bc")
                                nc.gpsimd.partition_broadcast(bc[:], rr[:])
                                nc.vector.tensor_tensor(
                                    oT_sb[b][lo : lo + DK, hp, qlo : qlo + 512],
                                    ov[0:DK, :],
                                    bc[:],
                                    Alu.mult,
                                )
                            do_inject()

                # p1(0) standalone (ACT idle -> QK bias-adds on ACT)
                p1_prefetch(0, split_first=first)
                if first:
                    late_consts()
                for c in p1_chunks(0, on_act=True):
                    c()
                # p2(0) with p1(1) injected (ACT busy -> QK on DVE)
                p1_prefetch(1)
                phase2(0, p1_chunks(1, on_act=False))
                # p2(1) with p3(0) injected (ACT busy -> y copies on DVE)
                phase2(1, p3_chunks(0, act_half=False))
                # p3(1) tail (ACT idle -> half the y copies on ACT)
                for c in p3_chunks(1, act_half=True):
                    c()

            for r in range(repeat):
                emit(first=(r == 0))
    nc.finalize()
    return nc


def _get_nc():
    global _NC
    if _NC is None:
        _NC = _build_nc()
    return _NC


def _f16(a):
    return np.ascontiguousarray(np.asarray(a, np.float32).astype(np.float16))


def make_in_maps(batch, attn_bias, Wq, bq, Wk, bk, Wv, bv, Wo, bo):
    batch = np.asarray(batch, np.float32)
    attn_bias = np.asarray(attn_bias, np.float32)
    Wq, Wk, Wv, Wo = (np.asarray(w, np.float32) for w in (Wq, Wk, Wv, Wo))
    bq, bk = np.asarray(bq, np.float32), np.asarray(bk, np.float32)
    expb = np.exp(attn_bias, dtype=np.float32)
    in_maps = []
    for c in range(NCORES):
        bg, hg = c // HPC, c % HPC
        ds = slice(hg * DPC, (hg + 1) * DPC)
        xT = batch[bg * BPC : (bg + 1) * BPC].reshape(SL, HID).T
        # (b, head, q, k) -> (b, hp, sub, q, k) -> (b, hp, k, sub, q)
        eb = expb[bg * BPC : (bg + 1) * BPC, hg * HPC : (hg + 1) * HPC]
        eb = eb.reshape(BPC, 2, 2, S, S).transpose(0, 1, 4, 2, 3)
        eb = eb.reshape(BPC, 2, ST, P, 2, S)
        in_maps.append(
            {
                "xT": _f16(xT),
                "wqT": _f16((SCALE * Wq[ds]).T),
                "wkT": _f16(Wk[ds].T),
                "wvT": _f16(Wv[ds].T),
                "bqk": np.ascontiguousarray(np.stack([SCALE * bq[ds], bk[ds]])),
                "woT": _f16(Wo[:, ds].T),
                "ebias": eb.astype(np.float16),
            }
        )
    return in_maps


def gather(results, corr):
    out = np.zeros((B, S, HID), np.float32)
    for bg in range(B // BPC):
        acc = np.zeros((SL, HID), np.float32)
        for c in range(bg * 4, bg * 4 + 4):
            acc += np.asarray(results[c]["y"], np.float32)
        out[bg * BPC : (bg + 1) * BPC] = acc.reshape(BPC, S, HID)
    out += corr[None, None, :]
    return out


LAST_RESULTS = None


def kernel(**inputs):
    global LAST_RESULTS
    import os
    from concourse import bass_utils

    nc = _get_nc()
    in_maps = make_in_maps(**inputs)
    kwargs = {}
    if os.environ.get("KERNEL_TRACE"):
        kwargs = dict(trace=True)
    res = bass_utils.run_bass_kernel_spmd(
        nc, in_maps, core_ids=list(range(NCORES)), **kwargs
    )
    LAST_RESULTS = res
    Wo = np.asarray(inputs["Wo"], np.float32)
    bv = np.asarray(inputs["bv"], np.float32)
    bo = np.asarray(inputs["bo"], np.float32)
    corr = Wo @ bv + bo
    return gather(res.results, corr)


# revision 20
# speedup vs baseline: 4.4170x; 4.4170x over previous
"""Multi-head attention (B=4, S=1024, HID=1024, NH=16) on 8 trn2 NeuronCores.

Sharding: core c handles batch group bg=c//4 (2 batches) and head group
hg=c%4 (4 heads, i.e. dim slice hg*256:(hg+1)*256).  Each core computes a
partial output y_c = o_heads @ Wo[:, slice].T of the full [2048, 1024] shape;
the host sums the 4 partials per batch group and adds the (bo + bv @ Wo.T)
correction row (softmax rows sum to 1, so the V bias commutes to the end).

Device dataflow (per core, all matmul operands fp16, fp32 PSUM accum):
  p1: qT/kT = W @ x.T (transposed layout, per-partition bias via ACT/DVE),
      v = x @ Wv.T with a ones column appended (softmax denominator rides
      the PV matmul as output row 64).
  p2: per (head-pair, q-tile, k-tile): scores via two K=64 matmuls at base
      partitions 0/64 (auto row tile_position -> they run concurrently in
      the PE array); exp on ACT over [128,1024] PSUM (both heads at once);
      e = exp(s) * expb on DVE in 2x mode (expb = exp(attn_bias) is
      precomputed on the host, fp16, so the bias-add becomes an all-16-bit
      SBUF multiply instead of a 1x f32 PSUM add); PV accumulates
      [65, 512] over k-tiles; normalize via DVE reciprocal + GpSimd
      partition_broadcast + DVE multiply.
  p3: y = oT.T @ woT, staged in SBUF, one 2MB DMA per batch (fp16 partials
      summed on the host).

Phase-interleaved emission: phase 2 is ACT-exp-bound, so the other batch's
projection/output-projection work is injected into phase 2's instruction
streams at 8 points, keeping every engine busy:
    p1(0) | p2(0)+p1(1) chunks | p2(1)+p3(0) chunks | p3(1)
All DMA is MB-scale (14 transfers/core): exp-bias 4MB per head-pair, x 2MB
per batch, y 2MB per batch.  Measured ~97us/core on trn2 (differential
repeat-NEFF timing), vs ~259us for the naive per-phase version.
"""

import numpy as np

B, S, HID, NH, DK = 4, 1024, 1024, 16, 64
SCALE = DK**-0.5
P = 128
NCORES = 8
HPC = 4  # heads per core
BPC = 2  # batches per core
SL = BPC * S  # 2048 local rows
DPC = HPC * DK  # 256 local head dims
KT = HID // P  # 8 contraction tiles for the projections
ST = S // P  # 8 seq tiles of 128

_NC = None


def _build_nc(repeat=1):
    import concourse.tile as tile
    from concourse import bacc, mybir
    from contextlib import ExitStack

    f32 = mybir.dt.float32
    f16 = mybir.dt.float16
    Alu = mybir.AluOpType
    Act = mybir.ActivationFunctionType

    nc = bacc.Bacc()

    xT_d = nc.dram_tensor("xT", [HID, SL], f16, kind="ExternalInput")
    wqT_d = nc.dram_tensor("wqT", [HID, DPC], f16, kind="ExternalInput")
    wkT_d = nc.dram_tensor("wkT", [HID, DPC], f16, kind="ExternalInput")
    wvT_d = nc.dram_tensor("wvT", [HID, DPC], f16, kind="ExternalInput")
    bqk_d = nc.dram_tensor("bqk", [2, DPC], f32, kind="ExternalInput")
    woT_d = nc.dram_tensor("woT", [DPC, HID], f16, kind="ExternalInput")
    # exp(bias), transposed+tiled on host: (b, hp, kt, p, sub, q)
    ebias_d = nc.dram_tensor(
        "ebias", [BPC, HPC // 2, ST, P, 2, S], f16, kind="ExternalInput"
    )
    y_d = nc.dram_tensor("y", [SL, HID], f16, kind="ExternalOutput")

    with tile.TileContext(nc) as tc:
        with ExitStack() as ctx:
            const = ctx.enter_context(tc.tile_pool(name="const", bufs=1))
            persist = ctx.enter_context(tc.tile_pool(name="persist", bufs=1))
            xchunk = ctx.enter_context(tc.tile_pool(name="xchunk", bufs=2))
            ebp = ctx.enter_context(tc.tile_pool(name="ebp", bufs=2))
            epool = ctx.enter_context(tc.tile_pool(name="epool", bufs=3))
            e2pool = ctx.enter_context(tc.tile_pool(name="e2pool", bufs=3))
            small = ctx.enter_context(tc.tile_pool(name="small", bufs=3))
            youtp = ctx.enter_context(tc.tile_pool(name="youtp", bufs=1))
            ps_a = ctx.enter_context(tc.tile_pool(name="ps_a", bufs=3, space="PSUM"))
            ps_pv = ctx.enter_context(tc.tile_pool(name="ps_pv", bufs=2, space="PSUM"))

            # ---- constants (wq first so the first QK matmul group can start
            # as soon as wq + the first xc half land) ----
            wq_sb = const.tile([P, KT, DPC], f16, tag="wq")
            wk_sb = const.tile([P, KT, DPC], f16, tag="wk")
            wv_sb = const.tile([P, KT, DPC], f16, tag="wv")
            wo_sb = const.tile([P, 2, HID], f16, tag="wo")
            bq_sb = const.tile([P, 2], f32, tag="bq")
            bk_sb = const.tile([P, 2], f32, tag="bk")
            xT_r = xT_d.rearrange("(kt p) s -> p kt s", p=P)
            nc.sync.dma_start(wq_sb[:], wqT_d.rearrange("(kt p) m -> p kt m", p=P))

            def late_consts():
                nc.sync.dma_start(bq_sb[:], bqk_d[0].rearrange("(m p) -> p m", p=P))
                nc.sync.dma_start(bk_sb[:], bqk_d[1].rearrange("(m p) -> p m", p=P))
                nc.sync.dma_start(wk_sb[:], wkT_d.rearrange("(kt p) m -> p kt m", p=P))
                nc.sync.dma_start(wv_sb[:], wvT_d.rearrange("(kt p) m -> p kt m", p=P))
                nc.sync.dma_start(wo_sb[:], woT_d.rearrange("(kt p) n -> p kt n", p=P))

            def emit(first):
                # ---- persistent activations, per batch ----
                qT_sb, kT_sb, v_sb, oT_sb, xc_sb = [], [], [], [], [None, None]
                for b in range(BPC):
                    q_t = persist.tile([P, 2, S], f16, tag=f"qT{b}", name="q_t")
                    k_t = persist.tile([P, 2, S], f16, tag=f"kT{b}", name="k_t")
                    v_t = persist.tile(
                        [P, ST, HPC, DK + 1], f16, tag=f"v{b}", name="v_t"
                    )
                    o_t = persist.tile([P, 2, S], f16, tag=f"oT{b}", name="o_t")
                    nc.vector.memset(v_t[:, :, :, DK : DK + 1], 1.0)
                    qT_sb.append(q_t)
                    kT_sb.append(k_t)
                    v_sb.append(v_t)
                    oT_sb.append(o_t)

                def p1_prefetch(b, split_first=False):
                    xc = xchunk.tile([P, KT, S], f16, tag="xc", name="xc")
                    for h in range(2):
                        nc.sync.dma_start(
                            xc[:, :, h * 512 : (h + 1) * 512],
                            xT_r[:, :, b * S + h * 512 : b * S + (h + 1) * 512],
                        )
                    xc_sb[b] = xc

                def p1_qk_m(b, w_sb, b_sb, dst, m, on_act):
                    xc = xc_sb[b]
                    ps = ps_a.tile([P, 1024], f32, tag="a", name="a")
                    for h in range(2):
                        for kt in range(KT):
                            nc.tensor.matmul(
                                ps[:, h * 512 : (h + 1) * 512],
                                lhsT=w_sb[:, kt, m * P : (m + 1) * P],
                                rhs=xc[:, kt, h * 512 : (h + 1) * 512],
                                start=(kt == 0),
                                stop=(kt == KT - 1),
                            )
                    if on_act:
                        nc.scalar.activation(
                            dst[:, m, :], ps[:], Act.Identity, bias=b_sb[:, m : m + 1]
                        )
                    else:
                        nc.vector.tensor_scalar_add(
                            dst[:, m, :], ps[:], b_sb[:, m : m + 1]
                        )

                def p1_v(b, sts):
                    xc = xc_sb[b]
                    for st in sts:
                        ps = ps_a.tile([P, 1024], f32, tag="a", name="a")
                        for kt in range(KT):
                            nc.tensor.matmul(
                                ps[:, :DPC],
                                lhsT=xc[:, kt, st * P : (st + 1) * P],
                                rhs=wv_sb[:, kt, :],
                                start=(kt == 0),
                                stop=(kt == KT - 1),
                            )
                        nc.vector.tensor_copy(
                            out=v_sb[b][:, st, :, 0:DK],
                            in_=ps[:, :DPC].rearrange("p (h d) -> p h d", h=HPC),
                        )

                def p1_chunks(b, on_act, interleaved=False):
                    qk = [
                        lambda: p1_qk_m(b, wq_sb, bq_sb, qT_sb[b], 0, on_act),
                        lambda: p1_qk_m(b, wq_sb, bq_sb, qT_sb[b], 1, on_act),
                        lambda: p1_qk_m(b, wk_sb, bk_sb, kT_sb[b], 0, on_act),
                        lambda: p1_qk_m(b, wk_sb, bk_sb, kT_sb[b], 1, on_act),
                    ]
                    v = [
                        lambda: p1_v(b, range(0, 2)),
                        lambda: p1_v(b, range(2, 4)),
                        lambda: p1_v(b, range(4, 6)),
                        lambda: p1_v(b, range(6, 8)),
                    ]
                    return qk + v

                def p3_sl(b, ysb, sl, on_act):
                    yp = ps_a.tile([P, 1024], f32, tag="a", name="a")
                    for nt in range(2):
                        for hp in range(2):
                            nc.tensor.matmul(
                                yp[:, nt * 512 : (nt + 1) * 512],
                                lhsT=oT_sb[b][:, hp, sl * P : (sl + 1) * P],
                                rhs=wo_sb[:, hp, nt * 512 : (nt + 1) * 512],
                                start=(hp == 0),
                                stop=(hp == 1),
                            )
                    if on_act:
                        nc.scalar.copy(ysb[:, sl, :], yp[:])
                    else:
                        nc.vector.tensor_copy(out=ysb[:, sl, :], in_=yp[:])

                def p3_chunks(b, act_half):
                    holder = {}

                    def chunk(i):
                        def run():
                            if i == 0:
                                holder["ysb"] = youtp.tile(
                                    [P, ST, HID], f16, tag="yt", name="yt"
                                )
                            ysb = holder["ysb"]
                            on_act = act_half and (i % 2 == 0)
                            p3_sl(b, ysb, i, on_act)
                            if i == ST - 1:
                                nc.sync.dma_start(
                                    y_d[b * S : (b + 1) * S, :].rearrange(
                                        "(sl p) n -> p sl n", p=P
                                    ),
                                    ysb[:],
                                )

                        return run

                    return [chunk(i) for i in range(ST)]

                def phase2(b, inject):
                    ii = [0]

                    def do_inject():
                        if inject is not None and ii[0] < len(inject):
                            inject[ii[0]]()
                            ii[0] += 1

                    for hp in range(2):  # head pair (partition halves)
                        eb = ebp.tile([P, ST, 2, S], f16, tag="eb", name="eb")
                        nc.sync.dma_start(
                            eb[:], ebias_d[b, hp].rearrange("kt p s q -> p kt s q")
                        )
                        for qt in range(2):  # 512-wide q tiles
                            qlo = qt * 512
                            ovs = []
                            for sub in range(2):
                                ov = ps_pv.tile([P, 512], f32, tag="pv", name="pv")
                                ovs.append(ov)
                            for kt in range(ST):
                                if kt == ST // 2:
                                    do_inject()
                                klo = kt * P
                                sc = ps_a.tile([P, 2, 512], f32, tag="a", name="a")
                                for sub in range(2):
                                    lo = sub * DK
                                    nc.tensor.matmul(
                                        sc[:, sub, :],
                                        lhsT=kT_sb[b][lo : lo + DK, hp, klo : klo + P],
                                        rhs=qT_sb[b][lo : lo + DK, hp, qlo : qlo + 512],
                                        start=True,
                                        stop=True,
                                    )
                                er = epool.tile([P, 2, 512], f16, tag="e", name="e")
                                nc.scalar.activation(er[:], sc[:], Act.Exp)
                                et = e2pool.tile([P, 2, 512], f16, tag="e2", name="e2")
                                nc.vector.tensor_tensor(
                                    et[:],
                                    er[:],
                                    eb[:, kt, :, qlo : qlo + 512],
                                    Alu.mult,
                                )
                                for sub in range(2):
                                    h = hp * 2 + sub
                                    nc.tensor.matmul(
                                        ovs[sub][0 : DK + 1, :],
                                        lhsT=v_sb[b][:, kt, h, :],
                                        rhs=et[:, sub, :],
                                        start=(kt == 0),
                                        stop=(kt == ST - 1),
                                    )
                            for sub in range(2):
                                lo = sub * DK
                                ov = ovs[sub]
                                # Copy ov out of PSUM first: releases the PSUM
                                # bank for the next q-tile's PV accumulation
                                # immediately, so the whole normalize chain
                                # below runs off the critical path (oT is not
                                # needed until phase 3).
                                ovc = small.tile(
                                    [DK + 1, 512], f32, tag="ovc", name="ovc"
                                )
                                nc.vector.tensor_copy(
                                    out=ovc[:], in_=ov[0 : DK + 1, :]
                                )
                                # 1/d as exp(-ln d) on ACT: the DVE reciprocal
                                # is a multi-pass op on a single lane (~3.2us
                                # per [1,512]); two ACT spline evals are much
                                # cheaper and Ln/Exp share one table set.
                                rl = small.tile([1, 512], f32, tag="rl", name="rl")
                                nc.scalar.activation(
                                    rl[:], ovc[DK : DK + 1, :], Act.Ln
                                )
                                rr = small.tile([1, 512], f32, tag="rr", name="rr")
                                nc.scalar.activation(rr[:], rl[:], Act.Exp, scale=-1.0)
                                bc = small.tile([DK, 512], f32, tag="bc", name="bc")
                                nc.gpsimd.partition_broadcast(bc[:], rr[:])
                                nc.vector.tensor_tensor(
                                    oT_sb[b][lo : lo + DK, hp, qlo : qlo + 512],
                                    ovc[0:DK, :],
                                    bc[:],
                                    Alu.mult,
                                )
                            do_inject()

                # p1(0) standalone (ACT idle -> QK bias-adds on ACT)
                p1_prefetch(0, split_first=first)
                if first:
                    late_consts()
                for c in p1_chunks(0, on_act=True):
                    c()
                # p2(0) with p1(1) injected (ACT busy -> QK on DVE)
                p1_prefetch(1)
                phase2(0, p1_chunks(1, on_act=False))
                # p2(1) with p3(0) injected (ACT busy -> y copies on DVE)
                phase2(1, p3_chunks(0, act_half=False))
                # p3(1) tail (ACT idle -> half the y copies on ACT)
                for c in p3_chunks(1, act_half=True):
                    c()

            for r in range(repeat):
                emit(first=(r == 0))
    nc.finalize()
    return nc


def _get_nc():
    global _NC
    if _NC is None:
        _NC = _build_nc()
    return _NC


def _f16(a):
    return np.ascontiguousarray(np.asarray(a, np.float32).astype(np.float16))


def make_in_maps(batch, attn_bias, Wq, bq, Wk, bk, Wv, bv, Wo, bo):
    batch = np.asarray(batch, np.float32)
    attn_bias = np.asarray(attn_bias, np.float32)
    Wq, Wk, Wv, Wo = (np.asarray(w, np.float32) for w in (Wq, Wk, Wv, Wo))
    bq, bk = np.asarray(bq, np.float32), np.asarray(bk, np.float32)
    expb = np.exp(attn_bias, dtype=np.float32)
    in_maps = []
    for c in range(NCORES):
        bg, hg = c // HPC, c % HPC
        ds = slice(hg * DPC, (hg + 1) * DPC)
        xT = batch[bg * BPC : (bg + 1) * BPC].reshape(SL, HID).T
        # (b, head, q, k) -> (b, hp, sub, q, k) -> (b, hp, k, sub, q)
        eb = expb[bg * BPC : (bg + 1) * BPC, hg * HPC : (hg + 1) * HPC]
        eb = eb.reshape(BPC, 2, 2, S, S).transpose(0, 1, 4, 2, 3)
        eb = eb.reshape(BPC, 2, ST, P, 2, S)
        in_maps.append(
            {
                "xT": _f16(xT),
                "wqT": _f16((SCALE * Wq[ds]).T),
                "wkT": _f16(Wk[ds].T),
                "wvT": _f16(Wv[ds].T),
                "bqk": np.ascontiguousarray(np.stack([SCALE * bq[ds], bk[ds]])),
                "woT": _f16(Wo[:, ds].T),
                "ebias": eb.astype(np.float16),
            }
        )
    return in_maps


def gather(results, corr):
    out = np.zeros((B, S, HID), np.float32)
    for bg in range(B // BPC):
        acc = np.zeros((SL, HID), np.float32)
        for c in range(bg * 4, bg * 4 + 4):
            acc += np.asarray(results[c]["y"], np.float32)
        out[bg * BPC : (bg + 1) * BPC] = acc.reshape(BPC, S, HID)
    out += corr[None, None, :]
    return out


LAST_RESULTS = None


def kernel(**inputs):
    global LAST_RESULTS
    import os
    from concourse import bass_utils

    nc = _get_nc()
    in_maps = make_in_maps(**inputs)
    kwargs = {}
    if os.environ.get("KERNEL_TRACE"):
        kwargs = dict(trace=True)
    res = bass_utils.run_bass_kernel_spmd(
        nc, in_maps, core_ids=list(range(NCORES)), **kwargs
    )
    LAST_RESULTS = res
    Wo = np.asarray(inputs["Wo"], np.float32)
    bv = np.asarray(inputs["bv"], np.float32)
    bo = np.asarray(inputs["bo"], np.float32)
    corr = Wo @ bv + bo
    return gather(res.results, corr)
